# revision 14
# baseline (speedup 1.0000x reference)
"""Trainium2 Bass kernel for AlarmworkRNN.

Key facts exploited:
  - The reference's z2 stream is dead code (output depends only on z1), so we
    only compute z1 = tanh(x_t @ W_in1.T + [t>=2] z1_prev @ W_rec1.T + b_in1)
    for t = 1..T-1 and the final tanh(z1_{T-1} @ W_out.T + b_out).
  - Pure batch data-parallelism: 256 batch rows -> 32 per NeuronCore.
  - State is held transposed+interleaved in SBUF: z[p, j*32+b] = z1[h=128j+p, b]
    so each step's matmul outputs are directly the next step's inputs.
  - Per step: one identity-matmul injects xproj_t into PSUM (start=True),
    64 bf16 matmuls (8 h'-chunks x 8 k-chunks) accumulate W_rec1 @ z, then
    one ScalarE tanh drains PSUM -> SBUF (bf16).
  - Input projections are computed on the PE in 16-step blocks, emitted
    between steps so they fill PE idle time during each tanh.
"""

import numpy as np
import ml_dtypes

import concourse.bass as bass
import concourse.bacc as bacc
import concourse.mybir as mybir
import concourse.tile as tile
from concourse.bass_utils import run_bass_kernel_spmd

BF16 = ml_dtypes.bfloat16

B, T_FULL, I, H, O = 256, 256, 512, 1024, 128
NCORES = 8
BS = B // NCORES          # 32 batch rows per core
TB = 16                   # timesteps per projection block
NJ = H // 128             # 8 output h' chunks
NK = H // 128             # 8 contraction chunks
NKI = I // 128            # 4 input contraction chunks


def _build(T):
    nc = bacc.Bacc("TRN2", target_bir_lowering=False, debug=False,
                   num_devices=NCORES)
    f32 = mybir.dt.float32
    bf16 = mybir.dt.bfloat16

    xt_d = nc.dram_tensor("xt", [NKI, 128, T * BS], bf16, kind="ExternalInput")
    wrt_d = nc.dram_tensor("wrt", [128, NK * NJ * 128], bf16, kind="ExternalInput")
    wit_d = nc.dram_tensor("wit", [128, NKI * NJ * 128], bf16, kind="ExternalInput")
    wot_d = nc.dram_tensor("wot", [128, NK * 128], bf16, kind="ExternalInput")
    id_d = nc.dram_tensor("ident", [128, 128], bf16, kind="ExternalInput")
    bin_d = nc.dram_tensor("bin", [128, NJ], f32, kind="ExternalInput")
    bout_d = nc.dram_tensor("bout", [128, 1], f32, kind="ExternalInput")
    out_d = nc.dram_tensor("out", [128, BS], f32, kind="ExternalOutput")

    nblocks = T // TB
    C = NJ * BS  # 256 state columns

    with tile.TileContext(nc) as tc:
        with (
            tc.tile_pool(name="const", bufs=1) as constp,
            tc.tile_pool(name="xproj", bufs=5) as xprojp,
            tc.tile_pool(name="state", bufs=3) as statep,
            tc.tile_pool(name="spsumA", bufs=2, space=bass.MemorySpace.PSUM) as spsumA,
            tc.tile_pool(name="spsumB", bufs=2, space=bass.MemorySpace.PSUM) as spsumB,
            tc.tile_pool(name="ppsum", bufs=3, space=bass.MemorySpace.PSUM) as ppsum,
            tc.tile_pool(name="outp", bufs=1) as outp,
        ):
            xt_sb = constp.tile([128, NKI * T * BS], bf16, tag="xt")
            for k in range(NKI):
                nc.sync.dma_start(
                    out=xt_sb[:, k * T * BS:(k + 1) * T * BS], in_=xt_d[:][k]
                )
            wrt_sb = constp.tile([128, NK * NJ * 128], bf16, tag="wrt")
            nc.sync.dma_start(out=wrt_sb[:], in_=wrt_d[:])
            wit_sb = constp.tile([128, NKI * NJ * 128], bf16, tag="wit")
            nc.sync.dma_start(out=wit_sb[:], in_=wit_d[:])
            wot_sb = constp.tile([128, NK * 128], bf16, tag="wot")
            nc.sync.dma_start(out=wot_sb[:], in_=wot_d[:])
            id_sb = constp.tile([128, 128], bf16, tag="ident")
            nc.sync.dma_start(out=id_sb[:], in_=id_d[:])
            bin_sb = constp.tile([128, NJ], f32, tag="bin")
            nc.sync.dma_start(out=bin_sb[:], in_=bin_d[:])
            bout_sb = constp.tile([128, 1], f32, tag="bout")
            nc.sync.dma_start(out=bout_sb[:], in_=bout_d[:])

            xproj_tiles = {}

            def proj_block_gen(n):
                """Emit projection for timesteps [n*TB, (n+1)*TB)."""
                xp = xprojp.tile([128, TB * C], bf16, tag="xproj")
                xproj_tiles[n] = xp
                t0 = n * TB
                for j in range(NJ):
                    ps = ppsum.tile([128, TB * BS], mybir.dt.float32, tag="pp")
                    for ki in range(NKI):
                        nc.tensor.matmul(
                            ps[:],
                            wit_sb[:, (ki * NJ + j) * 128:(ki * NJ + j + 1) * 128],
                            xt_sb[:, ki * T * BS + t0 * BS:
                                  ki * T * BS + (t0 + TB) * BS],
                            start=(ki == 0), stop=(ki == NKI - 1),
                        )
                        yield
                    # bias add + cast into interleaved (t, j, b) layout
                    xp_v = xp[:].rearrange("p (t c) -> p t c", c=C)
                    nc.vector.tensor_scalar_add(
                        xp_v[:, :, j * BS:(j + 1) * BS],
                        ps[:].rearrange("p (t b) -> p t b", b=BS),
                        bin_sb[:, j:j + 1],
                    )
                    yield

            gens = {}
            done = set()

            def pump(n, k=None):
                if n >= nblocks or n in done:
                    return
                if n not in gens:
                    gens[n] = proj_block_gen(n)
                g = gens[n]
                try:
                    if k is None:
                        while True:
                            next(g)
                    else:
                        for _ in range(k):
                            next(g)
                except StopIteration:
                    done.add(n)

            pump(0)

            HC = C // 2  # 128: columns per half (j-chunks 0..3 | 4..7)

            def rhs_k(zpair, k):
                # rhs slice for contraction chunk k from the (zA, zB) pair
                zA, zB = zpair
                if k < NK // 2:
                    return zA[:, k * BS:(k + 1) * BS]
                return zB[:, (k - NK // 2) * BS:(k - NK // 2 + 1) * BS]

            z_prev = None  # (zA, zB)
            for t in range(1, T):
                n = t // TB
                pump(n)      # ensure this step's block is fully emitted
                if t % TB == 8:
                    pump(n + 1)  # clump-emit next block (dense proj burst)

                psA = spsumA.tile([128, HC], mybir.dt.float32, tag="spA")
                psB = spsumB.tile([128, HC], mybir.dt.float32, tag="spB")
                xp = xproj_tiles[n]
                tt = t % TB
                nc.tensor.matmul(
                    psA[:], id_sb[:], xp[:, tt * C:tt * C + HC],
                    start=True, stop=(t == 1),
                )
                nc.tensor.matmul(
                    psB[:], id_sb[:], xp[:, tt * C + HC:(tt + 1) * C],
                    start=True, stop=(t == 1), skip_group_check=True,
                )
                if t >= 2:
                    # four blocks: (jlo,klo) (jhi,klo) (jlo,khi) (jhi,khi)
                    # k-first so this step can start on zA(t-1) alone; psA
                    # completes at end of block 3 -> tanh_A overlaps block 4.
                    for jh, kh in ((0, 0), (0, 1), (1, 0), (1, 1)):
                        ps = psA if jh == 0 else psB
                        for j in range(jh * 4, jh * 4 + 4):
                            for k in range(kh * 4, kh * 4 + 4):
                                nc.tensor.matmul(
                                    ps[:, (j - jh * 4) * BS:(j - jh * 4 + 1) * BS],
                                    wrt_sb[:, (k * NJ + j) * 128:
                                           (k * NJ + j + 1) * 128],
                                    rhs_k(z_prev, k),
                                    start=False,
                                    stop=(kh == 1 and j % 4 == 3 and k % 4 == 3),
                                    skip_group_check=True,
                                )
                zA = statep.tile([128, HC], mybir.dt.bfloat16, tag="za")
                zB = statep.tile([128, HC], mybir.dt.bfloat16, tag="zb")
                nc.scalar.activation(zA[:], psA[:], mybir.ActivationFunctionType.Tanh)
                nc.scalar.activation(zB[:], psB[:], mybir.ActivationFunctionType.Tanh)
                z_prev = (zA, zB)

            # output layer: out.T[o, b] = tanh(W_out @ z + b_out)
            ops_ = spsumA.tile([128, BS], mybir.dt.float32, tag="spA")
            for k in range(NK):
                nc.tensor.matmul(
                    ops_[:], wot_sb[:, k * 128:(k + 1) * 128],
                    rhs_k(z_prev, k),
                    start=(k == 0), stop=(k == NK - 1),
                )
            out_sb = outp.tile([128, BS], mybir.dt.float32, tag="out")
            nc.scalar.activation(
                out_sb[:], ops_[:], mybir.ActivationFunctionType.Tanh,
                bias=bout_sb[:, 0:1],
            )
            nc.sync.dma_start(out=out_d[:], in_=out_sb[:])

    nc.compile()
    return nc


def _prep_shared(W_in1, b_in1, W_rec1, W_out, b_out):
    wrt = (W_rec1.reshape(NJ, 128, NK, 128).transpose(3, 2, 0, 1)
           .reshape(128, NK * NJ * 128).astype(BF16))
    wit = (W_in1.reshape(NJ, 128, NKI, 128).transpose(3, 2, 0, 1)
           .reshape(128, NKI * NJ * 128).astype(BF16))
    wot = (W_out.reshape(128, NK, 128).transpose(2, 1, 0)
           .reshape(128, NK * 128).astype(BF16))
    ident = np.eye(128, dtype=np.float32).astype(BF16)
    bin_ = np.ascontiguousarray(b_in1.reshape(NJ, 128).T).astype(np.float32)
    bout = b_out.reshape(128, 1).astype(np.float32)
    return dict(wrt=wrt, wit=wit, wot=wot, ident=ident, bin=bin_, bout=bout)


def _prep_xt(Xc, T):
    # Xc: [BS, T, I] -> [NKI, 128, T*BS] with element [k, p, t*BS+b] = Xc[b,t,128k+p]
    return np.ascontiguousarray(Xc.transpose(2, 1, 0)).reshape(
        NKI, 128, T * BS).astype(BF16)


_NC_CACHE = {}


def _run(inputs, T=T_FULL, trace=False, **spmd_kwargs):
    X = np.asarray(inputs["X"], dtype=np.float32)
    shared = _prep_shared(
        np.asarray(inputs["W_in1"], dtype=np.float32),
        np.asarray(inputs["b_in1"], dtype=np.float32),
        np.asarray(inputs["W_rec1"], dtype=np.float32),
        np.asarray(inputs["W_out"], dtype=np.float32),
        np.asarray(inputs["b_out"], dtype=np.float32),
    )
    if T not in _NC_CACHE:
        _NC_CACHE[T] = _build(T)
    nc = _NC_CACHE[T]

    in_maps = []
    for c in range(NCORES):
        m = dict(shared)
        m["xt"] = _prep_xt(X[c * BS:(c + 1) * BS, :T], T)
        in_maps.append(m)

    res = run_bass_kernel_spmd(nc, in_maps, core_ids=list(range(NCORES)),
                               trace=trace, **spmd_kwargs)
    Y = np.empty((B, O), dtype=np.float32)
    for c in range(NCORES):
        Y[c * BS:(c + 1) * BS] = np.asarray(res.results[c]["out"]).T
    return Y, res


def kernel(**inputs):
    return _run(inputs)[0]


# revision 15
# speedup vs baseline: 1.0250x; 1.0250x over previous
"""Trainium2 Bass kernel for AlarmworkRNN.

Key facts exploited:
  - The reference's z2 stream is dead code (output depends only on z1), so we
    only compute z1 = tanh(x_t @ W_in1.T + [t>=2] z1_prev @ W_rec1.T + b_in1)
    for t = 1..T-1 and the final tanh(z1_{T-1} @ W_out.T + b_out).
  - Pure batch data-parallelism: 256 batch rows -> 32 per NeuronCore.
  - State is held transposed+interleaved in SBUF: z[p, j*32+b] = z1[h=128j+p, b]
    so each step's matmul outputs are directly the next step's inputs.
  - Per step: one identity-matmul injects xproj_t into PSUM (start=True),
    64 bf16 matmuls (8 h'-chunks x 8 k-chunks) accumulate W_rec1 @ z, then
    one ScalarE tanh drains PSUM -> SBUF (bf16).
  - Input projections are computed on the PE in 16-step blocks, emitted
    between steps so they fill PE idle time during each tanh.
"""

import numpy as np
import ml_dtypes

import concourse.bass as bass
import concourse.bacc as bacc
import concourse.mybir as mybir
import concourse.tile as tile
from concourse.bass_utils import run_bass_kernel_spmd

BF16 = ml_dtypes.bfloat16

B, T_FULL, I, H, O = 256, 256, 512, 1024, 128
NCORES = 8
BS = B // NCORES          # 32 batch rows per core
TB = 16                   # timesteps per projection block
NJ = H // 128             # 8 output h' chunks
NK = H // 128             # 8 contraction chunks
NKI = I // 128            # 4 input contraction chunks


def _build(T):
    nc = bacc.Bacc("TRN2", target_bir_lowering=False, debug=False,
                   num_devices=NCORES)
    f32 = mybir.dt.float32
    bf16 = mybir.dt.bfloat16

    xt_d = nc.dram_tensor("xt", [NKI, 128, T * BS], bf16, kind="ExternalInput")
    wrt_d = nc.dram_tensor("wrt", [128, NK * NJ * 128], bf16, kind="ExternalInput")
    wit_d = nc.dram_tensor("wit", [128, NKI * NJ * 128], bf16, kind="ExternalInput")
    wot_d = nc.dram_tensor("wot", [128, NK * 128], bf16, kind="ExternalInput")
    id_d = nc.dram_tensor("ident", [128, 128], bf16, kind="ExternalInput")
    bin_d = nc.dram_tensor("bin", [128, NJ], f32, kind="ExternalInput")
    bout_d = nc.dram_tensor("bout", [128, 1], f32, kind="ExternalInput")
    out_d = nc.dram_tensor("out", [128, BS], f32, kind="ExternalOutput")

    nblocks = T // TB
    C = NJ * BS  # 256 state columns

    with tile.TileContext(nc) as tc:
        with (
            tc.tile_pool(name="const", bufs=1) as constp,
            tc.tile_pool(name="xproj", bufs=5) as xprojp,
            tc.tile_pool(name="state", bufs=3) as statep,
            tc.tile_pool(name="spsumA", bufs=2, space=bass.MemorySpace.PSUM) as spsumA,
            tc.tile_pool(name="spsumB", bufs=2, space=bass.MemorySpace.PSUM) as spsumB,
            tc.tile_pool(name="ppsum", bufs=3, space=bass.MemorySpace.PSUM) as ppsum,
            tc.tile_pool(name="outp", bufs=1) as outp,
        ):
            xt_sb = constp.tile([128, NKI * T * BS], bf16, tag="xt")
            for k in range(NKI):
                nc.sync.dma_start(
                    out=xt_sb[:, k * T * BS:(k + 1) * T * BS], in_=xt_d[:][k]
                )
            wrt_sb = constp.tile([128, NK * NJ * 128], bf16, tag="wrt")
            nc.sync.dma_start(out=wrt_sb[:], in_=wrt_d[:])
            wit_sb = constp.tile([128, NKI * NJ * 128], bf16, tag="wit")
            nc.sync.dma_start(out=wit_sb[:], in_=wit_d[:])
            wot_sb = constp.tile([128, NK * 128], bf16, tag="wot")
            nc.sync.dma_start(out=wot_sb[:], in_=wot_d[:])
            id_sb = constp.tile([128, 128], bf16, tag="ident")
            nc.sync.dma_start(out=id_sb[:], in_=id_d[:])
            bin_sb = constp.tile([128, NJ], f32, tag="bin")
            nc.sync.dma_start(out=bin_sb[:], in_=bin_d[:])
            bout_sb = constp.tile([128, 1], f32, tag="bout")
            nc.sync.dma_start(out=bout_sb[:], in_=bout_d[:])

            xproj_tiles = {}

            def proj_block_gen(n):
                """Emit projection for timesteps [n*TB, (n+1)*TB)."""
                xp = xprojp.tile([128, TB * C], bf16, tag="xproj")
                xproj_tiles[n] = xp
                t0 = n * TB
                for j in range(NJ):
                    ps = ppsum.tile([128, TB * BS], mybir.dt.float32, tag="pp")
                    for ki in range(NKI):
                        nc.tensor.matmul(
                            ps[:],
                            wit_sb[:, (ki * NJ + j) * 128:(ki * NJ + j + 1) * 128],
                            xt_sb[:, ki * T * BS + t0 * BS:
                                  ki * T * BS + (t0 + TB) * BS],
                            start=(ki == 0), stop=(ki == NKI - 1),
                        )
                        yield
                    # bias add + cast into interleaved (t, j, b) layout
                    xp_v = xp[:].rearrange("p (t c) -> p t c", c=C)
                    nc.vector.tensor_scalar_add(
                        xp_v[:, :, j * BS:(j + 1) * BS],
                        ps[:].rearrange("p (t b) -> p t b", b=BS),
                        bin_sb[:, j:j + 1],
                    )
                    yield

            gens = {}
            done = set()

            def pump(n, k=None):
                if n >= nblocks or n in done:
                    return
                if n not in gens:
                    gens[n] = proj_block_gen(n)
                g = gens[n]
                try:
                    if k is None:
                        while True:
                            next(g)
                    else:
                        for _ in range(k):
                            next(g)
                except StopIteration:
                    done.add(n)

            pump(0)

            HC = C // 2  # 128: columns per half (j-chunks 0..3 | 4..7)

            def rhs_k(zpair, k):
                # rhs slice for contraction chunk k from the (zA, zB) pair
                zA, zB = zpair
                if k < NK // 2:
                    return zA[:, k * BS:(k + 1) * BS]
                return zB[:, (k - NK // 2) * BS:(k - NK // 2 + 1) * BS]

            z_prev = None  # (zA, zB)
            for t in range(1, T):
                n = t // TB
                pump(n)      # ensure this step's block is fully emitted
                if t % TB == 8:
                    pump(n + 1)  # clump-emit next block (dense proj burst)

                psA = spsumA.tile([128, HC], mybir.dt.float32, tag="spA")
                psB = spsumB.tile([128, HC], mybir.dt.float32, tag="spB")
                xp = xproj_tiles[n]
                tt = t % TB
                nc.tensor.matmul(
                    psA[:], id_sb[:], xp[:, tt * C:tt * C + HC],
                    start=True, stop=(t == 1),
                )
                nc.tensor.matmul(
                    psB[:], id_sb[:], xp[:, tt * C + HC:(tt + 1) * C],
                    start=True, stop=(t == 1), skip_group_check=True,
                )
                if t >= 2:
                    # four blocks: (jlo,klo) (jhi,klo) (jlo,khi) (jhi,khi)
                    # k-first so this step can start on zA(t-1) alone; psA
                    # completes at end of block 3 -> tanh_A overlaps block 4.
                    for jh, kh in ((0, 0), (1, 0), (0, 1), (1, 1)):
                        ps = psA if jh == 0 else psB
                        for j in range(jh * 4, jh * 4 + 4):
                            for k in range(kh * 4, kh * 4 + 4):
                                nc.tensor.matmul(
                                    ps[:, (j - jh * 4) * BS:(j - jh * 4 + 1) * BS],
                                    wrt_sb[:, (k * NJ + j) * 128:
                                           (k * NJ + j + 1) * 128],
                                    rhs_k(z_prev, k),
                                    start=False,
                                    stop=(kh == 1 and j % 4 == 3 and k % 4 == 3),
                                    skip_group_check=True,
                                )
                zA = statep.tile([128, HC], mybir.dt.bfloat16, tag="za")
                zB = statep.tile([128, HC], mybir.dt.bfloat16, tag="zb")
                nc.scalar.activation(zA[:], psA[:], mybir.ActivationFunctionType.Tanh)
                nc.scalar.activation(zB[:], psB[:], mybir.ActivationFunctionType.Tanh)
                z_prev = (zA, zB)

            # output layer: out.T[o, b] = tanh(W_out @ z + b_out)
            ops_ = spsumA.tile([128, BS], mybir.dt.float32, tag="spA")
            for k in range(NK):
                nc.tensor.matmul(
                    ops_[:], wot_sb[:, k * 128:(k + 1) * 128],
                    rhs_k(z_prev, k),
                    start=(k == 0), stop=(k == NK - 1),
                )
            out_sb = outp.tile([128, BS], mybir.dt.float32, tag="out")
            nc.scalar.activation(
                out_sb[:], ops_[:], mybir.ActivationFunctionType.Tanh,
                bias=bout_sb[:, 0:1],
            )
            nc.sync.dma_start(out=out_d[:], in_=out_sb[:])

    nc.compile()
    return nc


def _prep_shared(W_in1, b_in1, W_rec1, W_out, b_out):
    wrt = (W_rec1.reshape(NJ, 128, NK, 128).transpose(3, 2, 0, 1)
           .reshape(128, NK * NJ * 128).astype(BF16))
    wit = (W_in1.reshape(NJ, 128, NKI, 128).transpose(3, 2, 0, 1)
           .reshape(128, NKI * NJ * 128).astype(BF16))
    wot = (W_out.reshape(128, NK, 128).transpose(2, 1, 0)
           .reshape(128, NK * 128).astype(BF16))
    ident = np.eye(128, dtype=np.float32).astype(BF16)
    bin_ = np.ascontiguousarray(b_in1.reshape(NJ, 128).T).astype(np.float32)
    bout = b_out.reshape(128, 1).astype(np.float32)
    return dict(wrt=wrt, wit=wit, wot=wot, ident=ident, bin=bin_, bout=bout)


def _prep_xt(Xc, T):
    # Xc: [BS, T, I] -> [NKI, 128, T*BS] with element [k, p, t*BS+b] = Xc[b,t,128k+p]
    return np.ascontiguousarray(Xc.transpose(2, 1, 0)).reshape(
        NKI, 128, T * BS).astype(BF16)


_NC_CACHE = {}


def _run(inputs, T=T_FULL, trace=False, **spmd_kwargs):
    X = np.asarray(inputs["X"], dtype=np.float32)
    shared = _prep_shared(
        np.asarray(inputs["W_in1"], dtype=np.float32),
        np.asarray(inputs["b_in1"], dtype=np.float32),
        np.asarray(inputs["W_rec1"], dtype=np.float32),
        np.asarray(inputs["W_out"], dtype=np.float32),
        np.asarray(inputs["b_out"], dtype=np.float32),
    )
    if T not in _NC_CACHE:
        _NC_CACHE[T] = _build(T)
    nc = _NC_CACHE[T]

    in_maps = []
    for c in range(NCORES):
        m = dict(shared)
        m["xt"] = _prep_xt(X[c * BS:(c + 1) * BS, :T], T)
        in_maps.append(m)

    res = run_bass_kernel_spmd(nc, in_maps, core_ids=list(range(NCORES)),
                               trace=trace, **spmd_kwargs)
    Y = np.empty((B, O), dtype=np.float32)
    for c in range(NCORES):
        Y[c * BS:(c + 1) * BS] = np.asarray(res.results[c]["out"]).T
    return Y, res


def kernel(**inputs):
    return _run(inputs)[0]


# revision 16
# speedup vs baseline: 1.0284x; 1.0033x over previous
"""Trainium2 Bass kernel for AlarmworkRNN.

Key facts exploited:
  - The reference's z2 stream is dead code (output depends only on z1), so we
    only compute z1 = tanh(x_t @ W_in1.T + [t>=2] z1_prev @ W_rec1.T + b_in1)
    for t = 1..T-1 and the final tanh(z1_{T-1} @ W_out.T + b_out).
  - Pure batch data-parallelism: 256 batch rows -> 32 per NeuronCore.
  - State is held transposed+interleaved in SBUF: z[p, j*32+b] = z1[h=128j+p, b]
    so each step's matmul outputs are directly the next step's inputs.
  - Per step: identity-matmuls inject xproj_t into PSUM (start=True), then
    64 bf16 matmuls (8 h'-chunks x 8 k-chunks) accumulate W_rec1 @ z, with
    the step split into two half-accumulations (j-chunks 0..3 -> PSUM A,
    4..7 -> PSUM B, separate banks and separate zA/zB state tiles) ordered
    k-first, so each ScalarE tanh (~0.9us semaphore+activation chain)
    overlaps the opposite half's matmuls instead of serializing.
  - Input projections are computed on the PE in 16-step blocks as dense
    back-to-back bursts (scattered matmuls pace ~377ns vs ~215ns clumped).
"""

import numpy as np
import ml_dtypes

import concourse.bass as bass
import concourse.bacc as bacc
import concourse.mybir as mybir
import concourse.tile as tile
from concourse.bass_utils import run_bass_kernel_spmd

BF16 = ml_dtypes.bfloat16

B, T_FULL, I, H, O = 256, 256, 512, 1024, 128
NCORES = 8
BS = B // NCORES          # 32 batch rows per core
TB = 16                   # timesteps per projection block
NJ = H // 128             # 8 output h' chunks
NK = H // 128             # 8 contraction chunks
NKI = I // 128            # 4 input contraction chunks


def _build(T):
    nc = bacc.Bacc("TRN2", target_bir_lowering=False, debug=False,
                   num_devices=NCORES)
    f32 = mybir.dt.float32
    bf16 = mybir.dt.bfloat16

    xt_d = nc.dram_tensor("xt", [NKI, 128, T * BS], bf16, kind="ExternalInput")
    wrt_d = nc.dram_tensor("wrt", [128, NK * NJ * 128], bf16, kind="ExternalInput")
    wit_d = nc.dram_tensor("wit", [128, NKI * NJ * 128], bf16, kind="ExternalInput")
    wot_d = nc.dram_tensor("wot", [128, NK * 128], bf16, kind="ExternalInput")
    id_d = nc.dram_tensor("ident", [128, 128], bf16, kind="ExternalInput")
    bin_d = nc.dram_tensor("bin", [128, NJ], f32, kind="ExternalInput")
    bout_d = nc.dram_tensor("bout", [128, 1], f32, kind="ExternalInput")
    out_d = nc.dram_tensor("out", [128, BS], f32, kind="ExternalOutput")

    nblocks = T // TB
    C = NJ * BS  # 256 state columns

    with tile.TileContext(nc) as tc:
        with (
            tc.tile_pool(name="const", bufs=1) as constp,
            tc.tile_pool(name="xproj", bufs=5) as xprojp,
            tc.tile_pool(name="state", bufs=3) as statep,
            tc.tile_pool(name="spsumA", bufs=2, space=bass.MemorySpace.PSUM) as spsumA,
            tc.tile_pool(name="spsumB", bufs=2, space=bass.MemorySpace.PSUM) as spsumB,
            tc.tile_pool(name="ppsum", bufs=3, space=bass.MemorySpace.PSUM) as ppsum,
            tc.tile_pool(name="outp", bufs=1) as outp,
        ):
            xt_sb = constp.tile([128, NKI * T * BS], bf16, tag="xt")
            for k in range(NKI):
                nc.sync.dma_start(
                    out=xt_sb[:, k * T * BS:(k + 1) * T * BS], in_=xt_d[:][k]
                )
            wrt_sb = constp.tile([128, NK * NJ * 128], bf16, tag="wrt")
            nc.sync.dma_start(out=wrt_sb[:], in_=wrt_d[:])
            wit_sb = constp.tile([128, NKI * NJ * 128], bf16, tag="wit")
            nc.sync.dma_start(out=wit_sb[:], in_=wit_d[:])
            wot_sb = constp.tile([128, NK * 128], bf16, tag="wot")
            nc.sync.dma_start(out=wot_sb[:], in_=wot_d[:])
            id_sb = constp.tile([128, 128], bf16, tag="ident")
            nc.sync.dma_start(out=id_sb[:], in_=id_d[:])
            bin_sb = constp.tile([128, NJ], f32, tag="bin")
            nc.sync.dma_start(out=bin_sb[:], in_=bin_d[:])
            bout_sb = constp.tile([128, 1], f32, tag="bout")
            nc.sync.dma_start(out=bout_sb[:], in_=bout_d[:])

            xproj_tiles = {}

            def proj_block_gen(n):
                """Emit projection for timesteps [n*TB, (n+1)*TB)."""
                xp = xprojp.tile([128, TB * C], bf16, tag="xproj")
                xproj_tiles[n] = xp
                t0 = n * TB
                for j in range(NJ):
                    ps = ppsum.tile([128, TB * BS], mybir.dt.float32, tag="pp")
                    for ki in range(NKI):
                        nc.tensor.matmul(
                            ps[:],
                            wit_sb[:, (ki * NJ + j) * 128:(ki * NJ + j + 1) * 128],
                            xt_sb[:, ki * T * BS + t0 * BS:
                                  ki * T * BS + (t0 + TB) * BS],
                            start=(ki == 0), stop=(ki == NKI - 1),
                        )
                        yield
                    # bias add + cast into interleaved (t, j, b) layout
                    xp_v = xp[:].rearrange("p (t c) -> p t c", c=C)
                    nc.vector.tensor_scalar_add(
                        xp_v[:, :, j * BS:(j + 1) * BS],
                        ps[:].rearrange("p (t b) -> p t b", b=BS),
                        bin_sb[:, j:j + 1],
                    )
                    yield

            gens = {}
            done = set()

            def pump(n, k=None):
                if n >= nblocks or n in done:
                    return
                if n not in gens:
                    gens[n] = proj_block_gen(n)
                g = gens[n]
                try:
                    if k is None:
                        while True:
                            next(g)
                    else:
                        for _ in range(k):
                            next(g)
                except StopIteration:
                    done.add(n)

            pump(0)

            HC = C // 2  # 128: columns per half (j-chunks 0..3 | 4..7)

            def rhs_k(zpair, k):
                # rhs slice for contraction chunk k from the (zA, zB) pair
                zA, zB = zpair
                if k < NK // 2:
                    return zA[:, k * BS:(k + 1) * BS]
                return zB[:, (k - NK // 2) * BS:(k - NK // 2 + 1) * BS]

            z_prev = None  # (zA, zB)
            for t in range(1, T):
                n = t // TB
                pump(n)      # ensure this step's block is fully emitted
                if t % TB == 8:
                    pump(n + 1)  # clump-emit next block (dense proj burst)

                psA = spsumA.tile([128, HC], mybir.dt.float32, tag="spA")
                psB = spsumB.tile([128, HC], mybir.dt.float32, tag="spB")
                xp = xproj_tiles[n]
                tt = t % TB
                nc.tensor.matmul(
                    psA[:], id_sb[:], xp[:, tt * C:tt * C + HC],
                    start=True, stop=(t == 1),
                )
                nc.tensor.matmul(
                    psB[:], id_sb[:], xp[:, tt * C + HC:(tt + 1) * C],
                    start=True, stop=(t == 1), skip_group_check=True,
                )
                if t >= 2:
                    # four blocks: (jlo,klo) (jhi,klo) (jlo,khi) (jhi,khi)
                    # k-first so this step can start on zA(t-1) alone; psA
                    # completes at end of block 3 -> tanh_A overlaps block 4.
                    for jh, kh in ((0, 0), (1, 0), (0, 1), (1, 1)):
                        ps = psA if jh == 0 else psB
                        for j in range(jh * 4, jh * 4 + 4):
                            for k in range(kh * 4, kh * 4 + 4):
                                nc.tensor.matmul(
                                    ps[:, (j - jh * 4) * BS:(j - jh * 4 + 1) * BS],
                                    wrt_sb[:, (k * NJ + j) * 128:
                                           (k * NJ + j + 1) * 128],
                                    rhs_k(z_prev, k),
                                    start=False,
                                    stop=(kh == 1 and j % 4 == 3 and k % 4 == 3),
                                    skip_group_check=True,
                                )
                zA = statep.tile([128, HC], mybir.dt.bfloat16, tag="za")
                zB = statep.tile([128, HC], mybir.dt.bfloat16, tag="zb")
                nc.scalar.activation(zA[:], psA[:], mybir.ActivationFunctionType.Tanh)
                nc.scalar.activation(zB[:], psB[:], mybir.ActivationFunctionType.Tanh)
                z_prev = (zA, zB)

            # output layer: out.T[o, b] = tanh(W_out @ z + b_out)
            ops_ = spsumA.tile([128, BS], mybir.dt.float32, tag="spA")
            for k in range(NK):
                nc.tensor.matmul(
                    ops_[:], wot_sb[:, k * 128:(k + 1) * 128],
                    rhs_k(z_prev, k),
                    start=(k == 0), stop=(k == NK - 1),
                )
            out_sb = outp.tile([128, BS], mybir.dt.float32, tag="out")
            nc.scalar.activation(
                out_sb[:], ops_[:], mybir.ActivationFunctionType.Tanh,
                bias=bout_sb[:, 0:1],
            )
            nc.sync.dma_start(out=out_d[:], in_=out_sb[:])

    nc.compile()
    return nc


def _prep_shared(W_in1, b_in1, W_rec1, W_out, b_out):
    wrt = (W_rec1.reshape(NJ, 128, NK, 128).transpose(3, 2, 0, 1)
           .reshape(128, NK * NJ * 128).astype(BF16))
    wit = (W_in1.reshape(NJ, 128, NKI, 128).transpose(3, 2, 0, 1)
           .reshape(128, NKI * NJ * 128).astype(BF16))
    wot = (W_out.reshape(128, NK, 128).transpose(2, 1, 0)
           .reshape(128, NK * 128).astype(BF16))
    ident = np.eye(128, dtype=np.float32).astype(BF16)
    bin_ = np.ascontiguousarray(b_in1.reshape(NJ, 128).T).astype(np.float32)
    bout = b_out.reshape(128, 1).astype(np.float32)
    return dict(wrt=wrt, wit=wit, wot=wot, ident=ident, bin=bin_, bout=bout)


def _prep_xt(Xc, T):
    # Xc: [BS, T, I] -> [NKI, 128, T*BS] with element [k, p, t*BS+b] = Xc[b,t,128k+p]
    return np.ascontiguousarray(Xc.transpose(2, 1, 0)).reshape(
        NKI, 128, T * BS).astype(BF16)


_NC_CACHE = {}


def _run(inputs, T=T_FULL, trace=False, **spmd_kwargs):
    X = np.asarray(inputs["X"], dtype=np.float32)
    shared = _prep_shared(
        np.asarray(inputs["W_in1"], dtype=np.float32),
        np.asarray(inputs["b_in1"], dtype=np.float32),
        np.asarray(inputs["W_rec1"], dtype=np.float32),
        np.asarray(inputs["W_out"], dtype=np.float32),
        np.asarray(inputs["b_out"], dtype=np.float32),
    )
    if T not in _NC_CACHE:
        _NC_CACHE[T] = _build(T)
    nc = _NC_CACHE[T]

    in_maps = []
    for c in range(NCORES):
        m = dict(shared)
        m["xt"] = _prep_xt(X[c * BS:(c + 1) * BS, :T], T)
        in_maps.append(m)

    res = run_bass_kernel_spmd(nc, in_maps, core_ids=list(range(NCORES)),
                               trace=trace, **spmd_kwargs)
    Y = np.empty((B, O), dtype=np.float32)
    for c in range(NCORES):
        Y[c * BS:(c + 1) * BS] = np.asarray(res.results[c]["out"]).T
    return Y, res


def kernel(**inputs):
    return _run(inputs)[0]
